# revision 1
# baseline (speedup 1.0000x reference)
"""Trainium2 Bass kernel for nn_Attention_84585085927925 — Gram variant.

Reference (per batch element b, all fp32):
    qkv = x @ w_qkv.T ; q,k,v heads of 64 ; attn = sqrt(64) * q @ k.T (NO
    softmax) ; out = attn @ v ; out = out @ w_fc.T + b_fc

With no softmax the attention is associative, and k/v can be folded into
the weights via the Gram matrix of x:
    out_h = (s*q_h) @ (k_h.T @ v_h) = (s*q_h) @ (wk_h @ (x.T x) @ wv_h.T)
Per-core pipeline (one batch element per NeuronCore, 8 cores, no
collectives; e = output-feature axis, d = input-feature axis):
    qT   = (s*w_q).T-stationary @ xT       -> [768,1024]
    C    = x.T x  (x-stationary)           -> [768,768] (symmetric)
    T1   = C-stationary @ wv.T             -> [768,768]
    G    = wk.T-stationary @ T1 per pair   -> block-diag [128,128] per pair
    aoT  = G2-stationary @ qT per pair     -> [768,1024]
    outT = w_fc.T-stationary @ aoT + b_fc  -> [768,1024]
Host transposes x and outT. Matmuls run in float32r (~4x faster than
fp32, ~3e-4 relative error).
"""

import numpy as np

import concourse.bass as bass  # noqa: F401  (registers engine namespaces)
import concourse.mybir as mybir
import concourse.tile as tile
from concourse import bacc, bass_utils

F32 = mybir.dt.float32
F32R = mybir.dt.float32r

B, N, D, H = 8, 1024, 768, 12
HD = D // H            # 64
SCALE = float(np.sqrt(HD))
DT = D // 128          # 6  d-tiles
ET = D // 128          # 6  e-tiles
NT = N // 128          # 8  n(token)-tiles
NC2 = N // 512         # 2  512-wide token chunks
ECH = 384              # e-chunk that fits one PSUM bank with headroom
NPAIR = H // 2         # 6 head pairs


def _build_program():
    nc = bacc.Bacc(
        trn_type="TRN2", target_bir_lowering=False, debug=False, num_devices=B
    )
    xT_d = nc.dram_tensor("xT", [D, N], F32, kind="ExternalInput").ap()
    xN_d = nc.dram_tensor("xN", [N, D], F32, kind="ExternalInput").ap()
    wqT_d = nc.dram_tensor("wqT", [D, D], F32, kind="ExternalInput").ap()
    wkT_d = nc.dram_tensor("wkT", [D, D], F32, kind="ExternalInput").ap()
    wvT_d = nc.dram_tensor("wvT", [D, D], F32, kind="ExternalInput").ap()
    wfcT_d = nc.dram_tensor("wfcT", [D, D], F32, kind="ExternalInput").ap()
    bfc_d = nc.dram_tensor("bfc", [D], F32, kind="ExternalInput").ap()
    outT_d = nc.dram_tensor("outT", [D, N], F32, kind="ExternalOutput").ap()

    with tile.TileContext(nc) as tc:
        with tc.tile_pool(name="big", bufs=1) as big, \
             tc.tile_pool(name="wsp", bufs=3) as wsp, \
             tc.tile_pool(name="outsp", bufs=6) as outsp, \
             tc.tile_pool(name="ps", bufs=6, space="PSUM") as ps, \
             tc.tile_pool(name="psg", bufs=2, space="PSUM") as psg:

            xT_sb = big.tile([128, DT, N], F32R, name="xT_sb")
            xN_sb = big.tile([128, NT, D], F32R, name="xN_sb")
            qT_sb = big.tile([128, ET, N], F32R, name="qT_sb")
            c_sb = big.tile([128, DT, D], F32R, name="c_sb")
            t1_sb = big.tile([128, DT, D], F32R, name="t1_sb")
            ao_sb = big.tile([128, DT, N], F32R, name="ao_sb")
            g2_sb = big.tile([128, NPAIR, 128], F32R, name="g2_sb")
            bias_sb = big.tile([128, ET], F32, name="bias_sb")

            wq_r = wqT_d.rearrange("(o p) e -> p o e", p=128).bitcast(F32R)
            xT_r = xT_d.rearrange("(o p) n -> p o n", p=128).bitcast(F32R)
            xN_r = xN_d.rearrange("(o p) e -> p o e", p=128).bitcast(F32R)

            wq_tiles = []
            for et in range(ET):
                wq_t = wsp.tile([128, DT, 128], F32R, tag="w128", bufs=7,
                                name=f"wq_t{et}", uniquify=False)
                wq_tiles.append(wq_t)
            # first-needed data first: wq0 halves, xT n-half 0, then the rest
            for dh in range(2):
                dsl = slice(dh * 3, dh * 3 + 3)
                nc.sync.dma_start(wq_tiles[0][:, dsl, :], wq_r[:, dsl, 0:128])
            for dt in range(DT):
                nc.sync.dma_start(xT_sb[:, dt, 0:512], xT_r[:, dt, 0:512])
            for et in range(1, ET):
                for dh in range(2):
                    dsl = slice(dh * 3, dh * 3 + 3)
                    nc.sync.dma_start(wq_tiles[et][:, dsl, :],
                                      wq_r[:, dsl, et * 128:(et + 1) * 128])
            for dt in range(DT):
                nc.sync.dma_start(xT_sb[:, dt, 512:1024], xT_r[:, dt, 512:1024])
            for nt in range(NT):
                nc.sync.dma_start(xN_sb[:, nt, :], xN_r[:, nt, :])
            nc.sync.dma_start(bias_sb[:],
                              bfc_d.rearrange("(o p) -> p o", p=128))

            # ---- q.T projection: lhsT = wqT tile [d,e], rhs = xT chunk ----
            qt_chunks = [(0, 0, 256), (0, 256, 256),
                         (1, 0, 512), (2, 0, 512), (3, 0, 512),
                         (4, 0, 512), (5, 0, 512),
                         (0, 512, 512), (1, 512, 512), (2, 512, 512),
                         (3, 512, 512), (4, 512, 512), (5, 512, 512)]
            for et, off, width in qt_chunks:
                wq_t = wq_tiles[et]
                pt = ps.tile([128, 512], F32, tag="ps", name="pt_q")
                for dt in range(DT):
                    nc.tensor.matmul(
                        pt[:, :width],
                        wq_t[:, dt, :],
                        xT_sb[:, dt, off:off + width],
                        start=(dt == 0), stop=(dt == DT - 1),
                    )
                nc.vector.tensor_copy(
                    qT_sb[:, et, off:off + width], pt[:, :width]
                )

            # ---- C = x.T x : lhsT = x tile [n, d1], rhs = x [n, d2-chunk] --
            for ec in range(D // ECH):
                for d1t in range(DT):
                    pt = ps.tile([128, ECH], F32, tag="ps", name="pt_c")
                    for nt in range(NT):
                        nc.tensor.matmul(
                            pt[:],
                            xN_sb[:, nt, d1t * 128:(d1t + 1) * 128],
                            xN_sb[:, nt, ec * ECH:(ec + 1) * ECH],
                            start=(nt == 0), stop=(nt == NT - 1),
                        )
                    nc.vector.tensor_copy(
                        c_sb[:, d1t, ec * ECH:(ec + 1) * ECH], pt[:]
                    )

            # ---- T1 = C @ wv.T : lhsT = C tile (symmetric), rhs = wvT ----
            wv_r = wvT_d.rearrange("(o p) e -> p o e", p=128).bitcast(F32R)
            for ec in range(D // ECH):
                wv_t = wsp.tile([128, DT, ECH], F32R, tag="w384",
                                name=f"wv_t{ec}", uniquify=False)
                for dh in range(3):
                    dsl = slice(dh * 2, dh * 2 + 2)
                    nc.sync.dma_start(
                        wv_t[:, dsl, :],
                        wv_r[:, dsl, ec * ECH:(ec + 1) * ECH],
                    )
                for d1t in range(DT):
                    pt = ps.tile([128, ECH], F32, tag="ps", name="pt_t1")
                    for d2t in range(DT):
                        nc.tensor.matmul(
                            pt[:],
                            c_sb[:, d2t, d1t * 128:(d1t + 1) * 128],
                            wv_t[:, d2t, :],
                            start=(d2t == 0), stop=(d2t == DT - 1),
                        )
                    nc.vector.tensor_copy(
                        t1_sb[:, d1t, ec * ECH:(ec + 1) * ECH], pt[:]
                    )

            # ---- G = wk @ T1 per head pair, stored block-diagonal ----
            wk_r = wkT_d.rearrange("(o p) e -> p o e", p=128).bitcast(F32R)
            for t in range(NPAIR):
                wk_t = wsp.tile([128, DT, 128], F32R, tag="w128", bufs=7,
                                name=f"wk_t{t}", uniquify=False)
                nc.sync.dma_start(wk_t[:], wk_r[:, :, t * 128:(t + 1) * 128])
                gp = psg.tile([128, 128], F32, tag="psg", name="gp")
                for dt in range(DT):
                    nc.tensor.matmul(
                        gp[:],
                        wk_t[:, dt, :],
                        t1_sb[:, dt, t * 128:(t + 1) * 128],
                        start=(dt == 0), stop=(dt == DT - 1),
                    )
                nc.vector.tensor_scalar_mul(g2_sb[:, t, :], gp[:], 0.0)
                nc.vector.tensor_copy(g2_sb[0:64, t, 0:64], gp[0:64, 0:64])
                nc.vector.tensor_copy(g2_sb[64:128, t, 64:128],
                                      gp[64:128, 64:128])

            # ---- attn-out.T then fc, interleaved per 512-chunk ----
            wfc_r = wfcT_d.rearrange("(o p) e -> p o e", p=128).bitcast(F32R)
            wfc_tiles = []
            for et in range(ET):
                wfc_t = wsp.tile([128, DT, 128], F32R, tag="w128", bufs=7,
                                 name=f"wfc_t{et}", uniquify=False)
                wfc_tiles.append(wfc_t)
                nc.sync.dma_start(wfc_t[:], wfc_r[:, :, et * 128:(et + 1) * 128])
            for ic in range(NC2):
                for t in range(NPAIR):
                    pt = ps.tile([128, 512], F32, tag="ps", name="pt_ao")
                    nc.tensor.matmul(
                        pt[:],
                        g2_sb[:, t, :],
                        qT_sb[:, t, ic * 512:(ic + 1) * 512],
                        start=True, stop=True,
                    )
                    dst_ap = ao_sb[:, t, ic * 512:(ic + 1) * 512]
                    if t % 2 == 0:
                        nc.vector.tensor_copy(dst_ap, pt[:])
                    else:
                        nc.scalar.copy(dst_ap, pt[:])
                for et in range(ET):
                    wfc_t = wfc_tiles[et]
                    pt = ps.tile([128, 512], F32, tag="ps", name="pt_fc")
                    for dt in range(DT):
                        nc.tensor.matmul(
                            pt[:],
                            wfc_t[:, dt, :],
                            ao_sb[:, dt, ic * 512:(ic + 1) * 512],
                            start=(dt == 0), stop=(dt == DT - 1),
                        )
                    ot = outsp.tile([128, 512], F32, tag="ot", name="ot")
                    nc.scalar.add(ot[:], pt[:], bias_sb[:, et:et + 1])
                    last = (ic == NC2 - 1 and et == ET - 1)
                    nsplit = 4 if last else 1
                    w = 128 // nsplit
                    for ph in range(nsplit):
                        nc.sync.dma_start(
                            outT_d[et * 128 + ph * w:et * 128 + (ph + 1) * w,
                                   ic * 512:(ic + 1) * 512],
                            ot[ph * w:(ph + 1) * w, :],
                        )

    nc.compile()
    return nc


_NC_CACHE = None
LAST_EXEC_NS = None
LAST_RES = None


def kernel(x, w_qkv, w_fc, b_fc, _trace=False):
    global _NC_CACHE, LAST_EXEC_NS, LAST_RES
    x = np.asarray(x, dtype=np.float32)
    w_qkv = np.asarray(w_qkv, dtype=np.float32)
    w_fc = np.asarray(w_fc, dtype=np.float32)
    b_fc = np.asarray(b_fc, dtype=np.float32)

    if _NC_CACHE is None:
        _NC_CACHE = _build_program()
    nc = _NC_CACHE

    wqT = np.ascontiguousarray((SCALE * w_qkv[:D]).T)
    wkT = np.ascontiguousarray(w_qkv[D:2 * D].T)
    wvT = np.ascontiguousarray(w_qkv[2 * D:].T)
    wfcT = np.ascontiguousarray(w_fc.T)

    in_maps = []
    for b in range(B):
        in_maps.append({
            "xT": np.ascontiguousarray(x[b].T),
            "xN": np.ascontiguousarray(x[b]),
            "wqT": wqT, "wkT": wkT, "wvT": wvT, "wfcT": wfcT,
            "bfc": b_fc,
        })

    res = bass_utils.run_bass_kernel_spmd(
        nc, in_maps, core_ids=list(range(B)), trace=_trace
    )
    LAST_EXEC_NS = res.exec_time_ns
    LAST_RES = res
    out = np.stack([res.results[b]["outT"].T for b in range(B)])
    return np.ascontiguousarray(out.astype(np.float32))



# revision 5
# speedup vs baseline: 1.2754x; 1.2754x over previous
"""Trainium2 Bass kernel for nn_Attention_84585085927925 — Gram/M-path variant.

Reference (per batch element b, all fp32):
    qkv = x @ w_qkv.T ; q,k,v heads of 64 ; attn = sqrt(64) * q @ k.T (NO
    softmax) ; out = attn @ v ; out = out @ w_fc.T + b_fc

With no softmax the attention is linear in x, so the whole layer collapses
to out = x @ M + b_fc with a data-dependent [768,768] matrix M:
    C   = x.T x                       (symmetric, upper blocks + PE transpose)
    T1  = C @ wv.T                    [768, 768]
    G_h = s * wk_h @ C @ wv_h.T       per head (block-diag pairs, from T1)
    A   = per-pair G2T.T @ wfcT       [768, 768]
    M   = wq.T @ A                    [768, 768]
    out = x @ M + b_fc                computed as outT = M.T-stationary @ xT
One batch element per NeuronCore (8 cores, no collectives). All matmul
inputs fp16 (fp32 PSUM accumulation): ~7e-4 end-to-end max rel error.
"""

import numpy as np

import concourse.bass as bass  # noqa: F401  (registers engine namespaces)
import concourse.mybir as mybir
import concourse.tile as tile
from concourse import bacc, bass_utils

F32 = mybir.dt.float32
F16 = mybir.dt.float16

B, N, D, H = 8, 1024, 768, 12
HD = D // H            # 64
SCALE = float(np.sqrt(HD))
P = 128
DT = D // P            # 6  d-tiles
NT = N // P            # 8  n(token)-tiles
NPAIR = H // 2         # 6 head pairs


def _build_program():
    nc = bacc.Bacc(
        trn_type="TRN2", target_bir_lowering=False, debug=False, num_devices=B
    )
    xN_d = nc.dram_tensor("xN", [N, D], F16, kind="ExternalInput").ap()
    xT_d = nc.dram_tensor("xT", [D, N], F16, kind="ExternalInput").ap()
    wv_d = nc.dram_tensor("wvT", [D, D], F16, kind="ExternalInput").ap()
    wk_d = nc.dram_tensor("wkT8", [D, D], F16, kind="ExternalInput").ap()
    wq_d = nc.dram_tensor("wqN", [D, D], F16, kind="ExternalInput").ap()
    wfc_d = nc.dram_tensor("wfcT", [D, D], F16, kind="ExternalInput").ap()
    bfc_d = nc.dram_tensor("bfc", [D], F32, kind="ExternalInput").ap()
    id_d = nc.dram_tensor("ident", [P, P], F16, kind="ExternalInput").ap()
    outT_d = nc.dram_tensor("outT", [D, N], F16, kind="ExternalOutput").ap()

    xN_r = xN_d.rearrange("(o p) d -> p o d", p=P)
    xT_r = xT_d.rearrange("(o p) n -> p o n", p=P)
    wv_r = wv_d.rearrange("(o p) c -> p o c", p=P)
    wk_r = wk_d.rearrange("(o p) c -> p o c", p=P)
    wq_r = wq_d.rearrange("(o p) c -> p o c", p=P)
    wfc_r = wfc_d.rearrange("(o p) c -> p o c", p=P)
    outT_r = outT_d.rearrange("(o p) n -> p o n", p=P)

    with tile.TileContext(nc) as tc:
        with tc.tile_pool(name="big", bufs=1) as big, \
             tc.tile_pool(name="outsp", bufs=6) as outsp, \
             tc.tile_pool(name="psp", bufs=4, space="PSUM") as psp, \
             tc.tile_pool(name="psg", bufs=2, space="PSUM") as psg:

            xN_sb = big.tile([P, NT, D], F16, name="xN_sb")
            xT_sb = big.tile([P, DT, N], F16, name="xT_sb")
            wv_sb = big.tile([P, DT, D], F16, name="wv_sb")
            wk_sb = big.tile([P, DT, D], F16, name="wk_sb")
            wq_sb = big.tile([P, DT, D], F16, name="wq_sb")
            wfc_sb = big.tile([P, DT, D], F16, name="wfc_sb")
            c_sb = big.tile([P, DT, D], F16, name="c_sb")
            t1_sb = big.tile([P, DT, D], F16, name="t1_sb")
            g2t_sb = big.tile([P, NPAIR, P], F16, name="g2t_sb")
            a_sb = big.tile([P, NPAIR, D], F16, name="a_sb")
            m_sb = big.tile([P, DT, D], F16, name="m_sb")
            bias_sb = big.tile([P, DT], F32, name="bias_sb")
            id_sb = big.tile([P, P], F16, name="id_sb")

            # ---- DMA loads: xN first (C starts after tile 0 arrives) ----
            for cc in range(4):
                nc.sync.dma_start(xN_sb[:, 0, cc * 192:(cc + 1) * 192],
                                  xN_r[:, 0, cc * 192:(cc + 1) * 192])
            for nt in range(1, NT):
                for cc in range(2):
                    nc.sync.dma_start(xN_sb[:, nt, cc * 384:(cc + 1) * 384],
                                      xN_r[:, nt, cc * 384:(cc + 1) * 384])
            nc.sync.dma_start(id_sb[:], id_d)
            for dt in range(DT):
                nc.sync.dma_start(wv_sb[:, dt, :], wv_r[:, dt, :])
            for dt in range(DT):
                nc.sync.dma_start(wk_sb[:, dt, :], wk_r[:, dt, :])
            for dt in range(DT):
                nc.sync.dma_start(wfc_sb[:, dt, :], wfc_r[:, dt, :])
            for dt in range(DT):
                nc.sync.dma_start(wq_sb[:, dt, :], wq_r[:, dt, :])
            nc.sync.dma_start(bias_sb[:], bfc_d.rearrange("(o p) -> p o", p=P))
            for dt in range(DT):
                for hh in range(2):
                    nc.sync.dma_start(xT_sb[:, dt, hh * 512:(hh + 1) * 512],
                                      xT_r[:, dt, hh * 512:(hh + 1) * 512])
            nc.vector.memset(g2t_sb[:], 0.0)

            copy_engines = [nc.vector.tensor_copy, nc.scalar.copy]
            cp_i = 0

            def copy(dst, src):
                nonlocal cp_i
                copy_engines[cp_i % 2](dst, src)
                cp_i += 1

            # ---- C = x.T x, upper-triangular 128-blocks, nt-outer groups ----
            # row-tile a covers cols [128a, 768) in chunks <= 384 wide
            groups = [
                [(0, 0, 384), (0, 384, 384), (1, 128, 384), (1, 512, 256)],
                [(2, 256, 384), (2, 640, 128), (3, 384, 384)],
                [(4, 512, 256), (5, 640, 128)],
            ]
            for grp in groups:
                tiles = [psp.tile([P, 512], F32, tag="ps", name="pc")
                         for _ in grp]
                for nt in range(NT):
                    for (a, c0, w), pt in zip(grp, tiles):
                        nc.tensor.matmul(
                            pt[:, :w],
                            xN_sb[:, nt, a * P:(a + 1) * P],
                            xN_sb[:, nt, c0:c0 + w],
                            start=(nt == 0), stop=(nt == NT - 1),
                        )
                for (a, c0, w), pt in zip(grp, tiles):
                    copy(c_sb[:, a, c0:c0 + w], pt[:, :w])

            # ---- T1 = C @ wvT rows desc; PE-transpose lower C blocks ----
            # row a needs lhsT blocks (d2, a): for d2 > a transpose stored
            # (a, d2).  Emit transposes for row a-1 ahead of row a's matmuls.
            def emit_transposes(a):
                for b in range(a + 1, DT):
                    tp = psg.tile([P, P], F16, tag="ptr", bufs=2, name="tp")
                    nc.tensor.transpose(tp[:], c_sb[:, a, b * P:(b + 1) * P],
                                        id_sb[:])
                    nc.vector.tensor_copy(c_sb[:, b, a * P:(a + 1) * P], tp[:])

            emit_transposes(4)
            for idx, a in enumerate([5, 4, 3, 2, 1, 0]):
                if a >= 2:
                    emit_transposes(a - 2)
                for ch in range(2):
                    pt = psp.tile([P, 512], F32, tag="ps", name="pt1")
                    for d2t in range(DT):
                        nc.tensor.matmul(
                            pt[:, :384],
                            c_sb[:, d2t, a * P:(a + 1) * P],
                            wv_sb[:, d2t, ch * 384:(ch + 1) * 384],
                            start=(d2t == 0), stop=(d2t == DT - 1),
                        )
                    copy(t1_sb[:, a, ch * 384:(ch + 1) * 384], pt[:, :384])

            # ---- G2T per pair: [vf, kf] = sum_d T1[d, vf] wkT8[d, kf] ----
            for t in range(NPAIR):
                pg = psg.tile([P, P], F32, tag="pg", bufs=2, name="pg")
                for dt in range(DT):
                    nc.tensor.matmul(
                        pg[:],
                        t1_sb[:, dt, t * P:(t + 1) * P],
                        wk_sb[:, dt, t * P:(t + 1) * P],
                        start=(dt == 0), stop=(dt == DT - 1),
                    )
                nc.vector.tensor_copy(g2t_sb[0:64, t, 0:64], pg[0:64, 0:64])
                nc.scalar.copy(g2t_sb[64:128, t, 64:128], pg[64:128, 64:128])

            # ---- A[kf, e] = sum_vf G2T[vf, kf] wfcT[vf, e] per pair ----
            for t in range(NPAIR):
                for ch in range(2):
                    pa = psp.tile([P, 512], F32, tag="ps", name="pa")
                    nc.tensor.matmul(
                        pa[:, :384],
                        g2t_sb[:, t, :],
                        wfc_sb[:, t, ch * 384:(ch + 1) * 384],
                        start=True, stop=True,
                    )
                    copy(a_sb[:, t, ch * 384:(ch + 1) * 384], pa[:, :384])

            # ---- M[d, e] = sum_kf wq[kf, d] A[kf, e] ----
            for dtile in range(DT):
                for ch in range(2):
                    pm = psp.tile([P, 512], F32, tag="ps", name="pm")
                    for kft in range(DT):
                        nc.tensor.matmul(
                            pm[:, :384],
                            wq_sb[:, kft, dtile * P:(dtile + 1) * P],
                            a_sb[:, kft, ch * 384:(ch + 1) * 384],
                            start=(kft == 0), stop=(kft == DT - 1),
                        )
                    copy(m_sb[:, dtile, ch * 384:(ch + 1) * 384], pm[:, :384])

            # ---- outT[e, n] = sum_d M[d, e] xT[d, n] + b[e] ----
            for et in range(DT):
                for nch in range(2):
                    po = psp.tile([P, 512], F32, tag="ps", name="po")
                    for dt in range(DT):
                        nc.tensor.matmul(
                            po[:],
                            m_sb[:, dt, et * P:(et + 1) * P],
                            xT_sb[:, dt, nch * 512:(nch + 1) * 512],
                            start=(dt == 0), stop=(dt == DT - 1),
                        )
                    ot = outsp.tile([P, 512], F16, tag="ot", name="ot")
                    nc.scalar.add(ot[:], po[:], bias_sb[:, et:et + 1])
                    last = (et == DT - 1 and nch == 1)
                    nsplit = 4 if last else 1
                    w = P // nsplit
                    for ph in range(nsplit):
                        nc.sync.dma_start(
                            outT_r[ph * w:(ph + 1) * w, et,
                                   nch * 512:(nch + 1) * 512],
                            ot[ph * w:(ph + 1) * w, :],
                        )

    nc.compile()
    return nc


_NC_CACHE = None
LAST_EXEC_NS = None
LAST_RES = None


def kernel(x, w_qkv, w_fc, b_fc, _trace=False):
    global _NC_CACHE, LAST_EXEC_NS, LAST_RES
    x = np.asarray(x, dtype=np.float32)
    w_qkv = np.asarray(w_qkv, dtype=np.float32)
    w_fc = np.asarray(w_fc, dtype=np.float32)
    b_fc = np.asarray(b_fc, dtype=np.float32)

    if _NC_CACHE is None:
        _NC_CACHE = _build_program()
    nc = _NC_CACHE

    f16 = np.float16
    wqN = np.ascontiguousarray(w_qkv[:D]).astype(f16)
    wkT8 = np.ascontiguousarray((SCALE * w_qkv[D:2 * D]).T).astype(f16)
    wvT = np.ascontiguousarray(w_qkv[2 * D:].T).astype(f16)
    wfcT = np.ascontiguousarray(w_fc.T).astype(f16)
    ident = np.eye(P, dtype=f16)

    in_maps = []
    for b in range(B):
        in_maps.append({
            "xN": x[b].astype(f16),
            "xT": np.ascontiguousarray(x[b].T).astype(f16),
            "wvT": wvT, "wkT8": wkT8, "wqN": wqN, "wfcT": wfcT,
            "bfc": b_fc, "ident": ident,
        })

    res = bass_utils.run_bass_kernel_spmd(
        nc, in_maps, core_ids=list(range(B)), trace=_trace
    )
    LAST_EXEC_NS = res.exec_time_ns
    LAST_RES = res
    out = np.stack([res.results[b]["outT"].T.astype(np.float32)
                    for b in range(B)])
    return np.ascontiguousarray(out)
